# revision 3
# baseline (speedup 1.0000x reference)
"""Trainium2 Bass kernel for nn_BioTokenMucusSim (gnn_message_passing).

Layout: data-parallel over 8 cores (batch row per core). Per core, 2048
tokens are processed in 16 tiles of 128 tokens; tokens map to SBUF
partitions, each token's [32,32] W tile / [32,4] h tile lives along the
free dimension.

Math (per token, nb=32):
  Wz = W with zeroed diagonal
  inflow_c = sum_j Wz[i,j] * h[j,c]   (only channels P, G, mix=E+G+1.5L)
  tot = sum_j Wz[i,j]; r = 1/(tot+1e-8)
  h_new channel updates (clamped to [0,1])
  dist[i,j] = sqrt(sum_c (x_c[i]-x_c[j])^2) over the 4 new channels
  W_new = clip(0.95*Wz + (0.05*L_i + 0.05*L_j)*dist, 0, 1)  (diag stays 0)

Engines: DVE = products/reduces/affine chains, ACT = squares/sqrt,
GpSimd = diffs/broadcast adds, SP = DMA.
"""
import numpy as np

B, S, NB, NC_CH = 8, 2048, 32, 4
TOK_PER_CORE = S  # batch row per core
TILE = 128
NTILES = TOK_PER_CORE // TILE
N_CORES = 8

_compiled = {}


def _build():
    import concourse.bass as bass
    import concourse.mybir as mybir
    from concourse.tile import TileContext

    f32 = mybir.dt.float32
    alu = mybir.AluOpType
    act = mybir.ActivationFunctionType

    nc = bass.Bass("TRN2", target_bir_lowering=False, debug=False,
                   num_devices=N_CORES)
    hD = nc.declare_dram_parameter("h", [TOK_PER_CORE, NB * NC_CH], f32,
                                   isOutput=False)
    WD = nc.declare_dram_parameter("W", [TOK_PER_CORE, NB * NB], f32,
                                   isOutput=False)
    sD = nc.declare_dram_parameter("stim", [TOK_PER_CORE, NB], f32,
                                   isOutput=False)
    hoD = nc.declare_dram_parameter("h_new", [TOK_PER_CORE, NB * NC_CH], f32,
                                    isOutput=True)
    woD = nc.declare_dram_parameter("W_new", [TOK_PER_CORE, NB * NB], f32,
                                    isOutput=True)

    P = TILE

    with TileContext(nc) as tc:
        with (
            tc.tile_pool(name="wio", bufs=3) as wio,
            tc.tile_pool(name="hio", bufs=3) as hio,
            tc.tile_pool(name="big", bufs=2) as big,
            tc.tile_pool(name="sm", bufs=2) as sm,
        ):
            for t in range(NTILES):
                r0 = t * TILE

                Wt = wio.tile([P, NB * NB], f32, tag="Wt")
                nc.sync.dma_start(out=Wt[:, :], in_=WD[r0:r0 + P, :])
                ht = hio.tile([P, NB * NC_CH], f32, tag="ht")
                nc.sync.dma_start(out=ht[:, :], in_=hD[r0:r0 + P, :])
                st = hio.tile([P, NB], f32, tag="st")
                nc.sync.dma_start(out=st[:, :], in_=sD[r0:r0 + P, :])

                Wflat = Wt[:, :]
                W3 = Wflat.rearrange("p (i j) -> p i j", i=NB)
                diag = bass.AP(tensor=Wflat.tensor, offset=Wflat.offset,
                               ap=[list(Wflat.ap[0]), [NB + 1, NB]])
                nc.gpsimd.memset(diag, 0.0)

                hc = ht[:, :].rearrange("p (j c) -> p c j", c=NC_CH)
                hE, hP, hG, hL = (hc[:, c, :] for c in range(NC_CH))

                # mixed channel for L_new: mix = E + G + 1.5 L
                hmix = sm.tile([P, NB], f32, tag="hmix")
                nc.vector.scalar_tensor_tensor(
                    out=hmix[:, :], in0=hL, scalar=1.5, in1=hG,
                    op0=alu.mult, op1=alu.add)
                nc.vector.tensor_add(hmix[:, :], hmix[:, :], hE)

                # prod tiles: (c, i, j) with c in {P, G}, and mix separate
                prodPG = big.tile([P, 2, NB, NB], f32, tag="prodPG")
                inPG_h = sm.tile([P, 2 * NB], f32, tag="inPG_h")
                # pack hP, hG contiguously so one TT covers both channels
                nc.vector.tensor_copy(out=inPG_h[:, 0:NB], in_=hP)
                nc.vector.tensor_copy(out=inPG_h[:, NB:2 * NB], in_=hG)
                hPGb = (inPG_h[:, :].rearrange("p (c j) -> p c j", c=2)
                        .unsqueeze(2).broadcast_to((P, 2, NB, NB)))
                W4 = W3.unsqueeze(1).broadcast_to((P, 2, NB, NB))
                nc.vector.tensor_mul(prodPG[:, :, :, :], W4, hPGb)

                prodM = big.tile([P, NB, NB], f32, tag="prodM")
                hMb = hmix[:, :].unsqueeze(1).broadcast_to((P, NB, NB))
                nc.gpsimd.tensor_mul(prodM[:, :, :], W3, hMb)

                inflPG = sm.tile([P, 2, NB], f32, tag="inflPG")
                nc.vector.tensor_reduce(out=inflPG[:, :, :],
                                        in_=prodPG[:, :, :, :],
                                        axis=mybir.AxisListType.X, op=alu.add)
                inflM = sm.tile([P, NB], f32, tag="inflM")
                nc.vector.tensor_reduce(out=inflM[:, :], in_=prodM[:, :, :],
                                        axis=mybir.AxisListType.X, op=alu.add)
                totW = sm.tile([P, NB], f32, tag="totW")
                nc.vector.tensor_reduce(out=totW[:, :], in_=W3,
                                        axis=mybir.AxisListType.X, op=alu.add)

                # recip = 1/(tot + 1e-8); pre-scaled variants
                recip = sm.tile([P, NB], f32, tag="recip")
                nc.vector.tensor_scalar(out=recip[:, :], in0=totW[:, :],
                                        scalar1=1e-8, scalar2=None,
                                        op0=alu.add)
                nc.vector.reciprocal(out=recip[:, :], in_=recip[:, :])
                recip3 = sm.tile([P, NB], f32, tag="recip3")
                nc.vector.tensor_scalar(out=recip3[:, :], in0=recip[:, :],
                                        scalar1=0.3, scalar2=None,
                                        op0=alu.mult)
                recip2 = sm.tile([P, NB], f32, tag="recip2")
                nc.vector.tensor_scalar(out=recip2[:, :], in0=recip[:, :],
                                        scalar1=0.2, scalar2=None,
                                        op0=alu.mult)

                Pn3 = sm.tile([P, NB], f32, tag="Pn3")
                nc.vector.tensor_mul(Pn3[:, :], inflPG[:, 0, :], recip3[:, :])
                Gn2 = sm.tile([P, NB], f32, tag="Gn2")
                nc.vector.tensor_mul(Gn2[:, :], inflPG[:, 1, :], recip2[:, :])
                Mix2 = sm.tile([P, NB], f32, tag="Mix2")
                nc.vector.tensor_mul(Mix2[:, :], inflM[:, :], recip2[:, :])

                hout = hio.tile([P, NB * NC_CH], f32, tag="hout")
                hoc = hout[:, :].rearrange("p (i c) -> p c i", c=NC_CH)
                tmp = sm.tile([P, NB], f32, tag="tmp")
                tmp2 = sm.tile([P, NB], f32, tag="tmp2")

                # E_new = clip(E + 0.3 stim - 0.4 P - 0.2 G)
                nc.vector.scalar_tensor_tensor(out=tmp[:, :], in0=st[:, :],
                                               scalar=0.3, in1=hE,
                                               op0=alu.mult, op1=alu.add)
                nc.vector.scalar_tensor_tensor(out=tmp[:, :], in0=hP,
                                               scalar=-0.4, in1=tmp[:, :],
                                               op0=alu.mult, op1=alu.add)
                nc.vector.scalar_tensor_tensor(out=tmp[:, :], in0=hG,
                                               scalar=-0.2, in1=tmp[:, :],
                                               op0=alu.mult, op1=alu.add)
                nc.vector.tensor_scalar(out=hoc[:, 0, :], in0=tmp[:, :],
                                        scalar1=0.0, scalar2=1.0,
                                        op0=alu.max, op1=alu.min)

                # P_new = clip(0.7 P + 0.5 stim + 0.3 Pn - 0.2 E)
                nc.vector.scalar_tensor_tensor(out=tmp[:, :], in0=hP,
                                               scalar=0.7, in1=Pn3[:, :],
                                               op0=alu.mult, op1=alu.add)
                nc.vector.scalar_tensor_tensor(out=tmp[:, :], in0=hE,
                                               scalar=-0.2, in1=tmp[:, :],
                                               op0=alu.mult, op1=alu.add)
                nc.vector.scalar_tensor_tensor(out=tmp[:, :], in0=st[:, :],
                                               scalar=0.5, in1=tmp[:, :],
                                               op0=alu.mult, op1=alu.add)
                nc.vector.tensor_scalar(out=hoc[:, 1, :], in0=tmp[:, :],
                                        scalar1=0.0, scalar2=1.0,
                                        op0=alu.max, op1=alu.min)

                # G_new = clip(0.8 G + 0.4 E - 0.4 E*P - 0.3 P + Gn2)
                nc.vector.tensor_mul(tmp2[:, :], hE, hP)
                nc.vector.scalar_tensor_tensor(out=tmp[:, :], in0=tmp2[:, :],
                                               scalar=-0.4, in1=Gn2[:, :],
                                               op0=alu.mult, op1=alu.add)
                nc.vector.scalar_tensor_tensor(out=tmp[:, :], in0=hP,
                                               scalar=-0.3, in1=tmp[:, :],
                                               op0=alu.mult, op1=alu.add)
                nc.vector.scalar_tensor_tensor(out=tmp[:, :], in0=hG,
                                               scalar=0.8, in1=tmp[:, :],
                                               op0=alu.mult, op1=alu.add)
                nc.vector.scalar_tensor_tensor(out=tmp[:, :], in0=hE,
                                               scalar=0.4, in1=tmp[:, :],
                                               op0=alu.mult, op1=alu.add)
                nc.vector.tensor_scalar(out=hoc[:, 2, :], in0=tmp[:, :],
                                        scalar1=0.0, scalar2=1.0,
                                        op0=alu.max, op1=alu.min)

                # L_new = clip(0.7 L - 0.3 P + Mix2)
                nc.vector.scalar_tensor_tensor(out=tmp[:, :], in0=hL,
                                               scalar=0.7, in1=Mix2[:, :],
                                               op0=alu.mult, op1=alu.add)
                nc.vector.scalar_tensor_tensor(out=tmp[:, :], in0=hP,
                                               scalar=-0.3, in1=tmp[:, :],
                                               op0=alu.mult, op1=alu.add)
                nc.vector.tensor_scalar(out=hoc[:, 3, :], in0=tmp[:, :],
                                        scalar1=0.0, scalar2=1.0,
                                        op0=alu.max, op1=alu.min)

                nc.sync.dma_start(out=hoD[r0:r0 + P, :], in_=hout[:, :])

                # Lh = 0.05 * L_new
                Lh = sm.tile([P, NB], f32, tag="Lh")
                nc.vector.tensor_scalar(out=Lh[:, :], in0=hoc[:, 3, :],
                                        scalar1=0.05, scalar2=None,
                                        op0=alu.mult)

                # pairwise diffs over the 4 channels; (c,i,j) layout
                d01 = big.tile([P, 2, NB, NB], f32, tag="d01")
                d23 = big.tile([P, 2, NB, NB], f32, tag="d23")
                for cset, dt_, eng in ((0, d01, nc.vector), (2, d23, nc.gpsimd)):
                    for k in range(2):
                        c = cset + k
                        xv = hoc[:, c, :]
                        xi = xv.unsqueeze(2).broadcast_to((P, NB, NB))
                        xj = xv.unsqueeze(1).broadcast_to((P, NB, NB))
                        eng.tensor_sub(dt_[:, k, :, :], xi, xj)

                q01 = big.tile([P, 2, NB, NB], f32, tag="q01")
                q23 = big.tile([P, 2, NB, NB], f32, tag="q23")
                nc.scalar.activation(out=q01[:, :, :, :], in_=d01[:, :, :, :],
                                     func=act.Square)
                nc.scalar.activation(out=q23[:, :, :, :], in_=d23[:, :, :, :],
                                     func=act.Square)

                s01 = big.tile([P, NB, NB], f32, tag="s01")
                nc.vector.tensor_add(s01[:, :, :], q01[:, 0, :, :],
                                     q01[:, 1, :, :])
                s23 = big.tile([P, NB, NB], f32, tag="s23")
                nc.gpsimd.tensor_add(s23[:, :, :], q23[:, 0, :, :],
                                     q23[:, 1, :, :])
                sq = big.tile([P, NB, NB], f32, tag="sq")
                nc.vector.tensor_add(sq[:, :, :], s01[:, :, :], s23[:, :, :])

                dist = big.tile([P, NB, NB], f32, tag="dist")
                nc.scalar.activation(out=dist[:, :, :], in_=sq[:, :, :],
                                     func=act.Sqrt)

                # ml = Lh_i + Lh_j ; t2 = ml * dist
                ml = big.tile([P, NB, NB], f32, tag="ml")
                Li = Lh[:, :].unsqueeze(2).broadcast_to((P, NB, NB))
                Lj = Lh[:, :].unsqueeze(1).broadcast_to((P, NB, NB))
                nc.gpsimd.tensor_add(ml[:, :, :], Li, Lj)
                t2 = big.tile([P, NB, NB], f32, tag="t2")
                nc.gpsimd.tensor_mul(t2[:, :, :], ml[:, :, :], dist[:, :, :])

                # W_new = clip(0.95*Wz + t2)
                wn = wio.tile([P, NB * NB], f32, tag="wn")
                wn3 = wn[:, :].rearrange("p (i j) -> p i j", i=NB)
                nc.vector.scalar_tensor_tensor(out=wn3, in0=W3, scalar=0.95,
                                               in1=t2[:, :, :],
                                               op0=alu.mult, op1=alu.add)
                nc.vector.tensor_scalar(out=wn[:, :], in0=wn[:, :],
                                        scalar1=0.0, scalar2=1.0,
                                        op0=alu.max, op1=alu.min)
                nc.sync.dma_start(out=woD[r0:r0 + P, :], in_=wn[:, :])

    _split_excess_waits(nc, mybir)
    return nc


def _split_excess_waits(nc, mybir, max_waits=1):
    """This walrus build encodes at most one sync-wait command per
    instruction. Hoist excess waits onto same-engine InstNoOp carriers
    inserted just before the offending instruction (engines execute their
    streams in order, so semantics are unchanged)."""
    carrier_id = [0]
    for fn in nc.m.functions:
        for blk in fn.blocks:
            insts = blk.instructions
            out = []
            changed = False
            for inst in insts:
                si = inst.sync_info
                waits = list(si.on_wait) if si and si.on_wait else []
                if len(waits) > max_waits:
                    changed = True
                    keep = waits[-max_waits:]
                    for w in waits[:-max_waits]:
                        carrier_id[0] += 1
                        nop = mybir.InstNoOp(
                            name=f"I-waitcarrier-{carrier_id[0]}", ins=[],
                            outs=[])
                        nop.engine = inst.engine
                        nop.sync_info = mybir.SyncInfo(on_wait=[w],
                                                       on_update=[])
                        out.append(nop)
                    si.on_wait = keep
                out.append(inst)
            if changed:
                blk.instructions = out


def _get_nc():
    if "nc" not in _compiled:
        _compiled["nc"] = _build()
    return _compiled["nc"]


def kernel(h, W, stim):
    from concourse.bass_utils import run_bass_kernel_spmd

    h = np.ascontiguousarray(h, dtype=np.float32)
    W = np.ascontiguousarray(W, dtype=np.float32)
    stim = np.ascontiguousarray(stim, dtype=np.float32)

    nc = _get_nc()
    in_maps = [
        {
            "h": h[c].reshape(TOK_PER_CORE, NB * NC_CH),
            "W": W[c].reshape(TOK_PER_CORE, NB * NB),
            "stim": stim[c].reshape(TOK_PER_CORE, NB),
        }
        for c in range(N_CORES)
    ]
    res = run_bass_kernel_spmd(nc, in_maps, list(range(N_CORES)))
    h_new = np.stack([
        res.results[c]["h_new"].reshape(S, NB, NC_CH) for c in range(N_CORES)
    ])
    W_new = np.stack([
        res.results[c]["W_new"].reshape(S, NB, NB) for c in range(N_CORES)
    ])
    return h_new, W_new


# revision 5
# speedup vs baseline: 1.0270x; 1.0270x over previous
"""Trainium2 Bass kernel for nn_BioTokenMucusSim (gnn_message_passing).

Layout: data-parallel over 8 cores (batch row per core). Per core, 2048
tokens are processed in 16 tiles of 128 tokens; tokens map to SBUF
partitions, each token's [32,32] W tile / [32,4] h tile lives along the
free dimension. Tiles are processed in groups of 4 so the small
per-[128,32] elementwise chains run once per group on [128,128] views.

Math (per token, nb=32):
  Wz = W with zeroed diagonal
  inflow_c = sum_j Wz[i,j] * h[j,c]   (channels P, G, mix=E+G+1.5L)
  tot = sum_j Wz[i,j]; r = 1/(tot+1e-8)
  h_new channel updates (clamped to [0,1])
  dist[i,j] = sqrt(sum_c (x_c[i]-x_c[j])^2) over the 4 new channels
  W_new = clip(0.95*Wz + (0.05*L_i + 0.05*L_j)*dist, 0, 1) (diag stays 0)

Engines: DVE = products/reduces/affine chains, ACT = squares/sqrt,
GpSimd = diffs/broadcast adds, SP = DMA.
"""
import numpy as np

B, S, NB, NCH = 8, 2048, 32, 4
TOK_PER_CORE = S
TILE = 128
NTILES = TOK_PER_CORE // TILE  # 16
GRP = 4                        # tiles per small-op group
NGRP = NTILES // GRP
N_CORES = 8
HB = NB * NCH                  # 128 floats of h per token

_compiled = {}


def _build():
    import concourse.bass as bass
    import concourse.mybir as mybir
    from concourse.tile import TileContext

    f32 = mybir.dt.float32
    alu = mybir.AluOpType
    act = mybir.ActivationFunctionType

    nc = bass.Bass("TRN2", target_bir_lowering=False, debug=False,
                   num_devices=N_CORES)
    hD = nc.declare_dram_parameter("h", [TOK_PER_CORE, HB], f32,
                                   isOutput=False)
    WD = nc.declare_dram_parameter("W", [TOK_PER_CORE, NB * NB], f32,
                                   isOutput=False)
    sD = nc.declare_dram_parameter("stim", [TOK_PER_CORE, NB], f32,
                                   isOutput=False)
    hoD = nc.declare_dram_parameter("h_new", [TOK_PER_CORE, HB], f32,
                                    isOutput=True)
    woD = nc.declare_dram_parameter("W_new", [TOK_PER_CORE, NB * NB], f32,
                                    isOutput=True)

    P = TILE

    def stt(eng, out, in0, scalar, in1):
        eng.scalar_tensor_tensor(out=out, in0=in0, scalar=float(scalar),
                                 in1=in1, op0=alu.mult, op1=alu.add)

    def clip01(eng, out, in0):
        eng.tensor_scalar(out=out, in0=in0, scalar1=0.0, scalar2=1.0,
                          op0=alu.max, op1=alu.min)

    with TileContext(nc) as tc:
        with (
            tc.tile_pool(name="wio", bufs=8) as wio,
            tc.tile_pool(name="grp", bufs=2) as grp,
            tc.tile_pool(name="big", bufs=3) as big,
        ):
            for g in range(NGRP):
                g0 = g * GRP * TILE

                # ---- group-wide tiles ----
                hG = grp.tile([P, GRP, HB], f32, tag="hG")
                stG = grp.tile([P, GRP, NB], f32, tag="stG")
                houtG = grp.tile([P, GRP, HB], f32, tag="houtG")
                inflG = grp.tile([P, GRP, 3, NB], f32, tag="inflG")
                totG = grp.tile([P, GRP, NB], f32, tag="totG")
                LhG = grp.tile([P, GRP, NB], f32, tag="LhG")
                # per-group channel views of h: [p, t, c, j] with c selected
                hcG = hG[:, :, :].rearrange("p t (j c) -> p c t j", c=NCH)
                hEg, hPg, hGg, hLg = (hcG[:, c, :, :] for c in range(NCH))
                # h_new channel views: [p, t, c, i]
                hocG = houtG[:, :, :].rearrange("p t (i c) -> p c t i", c=NCH)

                Wts = []
                for k in range(GRP):
                    r0 = g0 + k * TILE

                    Wt = wio.tile([P, NB * NB], f32, tag="Wt")
                    Wts.append(Wt)
                    nc.sync.dma_start(out=Wt[:, :], in_=WD[r0:r0 + P, :])
                    nc.sync.dma_start(out=hG[:, k, :], in_=hD[r0:r0 + P, :])
                    nc.sync.dma_start(out=stG[:, k, :], in_=sD[r0:r0 + P, :])

                    Wflat = Wt[:, :]
                    W3 = Wflat.rearrange("p (i j) -> p i j", i=NB)
                    diag = bass.AP(tensor=Wflat.tensor, offset=Wflat.offset,
                                   ap=[list(Wflat.ap[0]), [NB + 1, NB]])
                    nc.gpsimd.memset(diag, 0.0)

                    # mix = E + G + 1.5 L  (small, per tile)
                    hmix = big.tile([P, NB], f32, tag="hmix")
                    stt(nc.vector, hmix[:, :], hcG[:, 3, k, :], 1.5,
                        hcG[:, 1 + 1, k, :])
                    nc.vector.tensor_add(hmix[:, :], hmix[:, :],
                                         hcG[:, 0, k, :])

                    # products for channels (P, G, mix) in one tile
                    prod3 = big.tile([P, 3, NB, NB], f32, tag="prod3")
                    hPGpack = big.tile([P, 2, NB], f32, tag="hPGpack")
                    nc.vector.tensor_copy(out=hPGpack[:, 0, :], in_=hPg[:, k, :])
                    nc.vector.tensor_copy(out=hPGpack[:, 1, :], in_=hGg[:, k, :])
                    hPGb = (hPGpack[:, :, :].unsqueeze(2)
                            .broadcast_to((P, 2, NB, NB)))
                    W4 = W3.unsqueeze(1).broadcast_to((P, 2, NB, NB))
                    nc.vector.tensor_mul(prod3[:, 0:2, :, :], W4, hPGb)
                    hMb = hmix[:, :].unsqueeze(1).broadcast_to((P, NB, NB))
                    nc.gpsimd.tensor_mul(prod3[:, 2, :, :], W3, hMb)

                    nc.vector.tensor_reduce(out=inflG[:, k, :, :],
                                            in_=prod3[:, :, :, :],
                                            axis=mybir.AxisListType.X,
                                            op=alu.add)
                    nc.vector.tensor_reduce(out=totG[:, k, :], in_=W3,
                                            axis=mybir.AxisListType.X,
                                            op=alu.add)

                # ---- group-wide neighbor means + h updates ([p, GRP*NB]) ----
                GT = GRP * NB
                tot2 = totG[:, :, :].rearrange("p t i -> p (t i)")
                recip = grp.tile([P, GT], f32, tag="recip")
                nc.vector.tensor_scalar(out=recip[:, :], in0=tot2,
                                        scalar1=1e-8, scalar2=None,
                                        op0=alu.add)
                nc.vector.reciprocal(out=recip[:, :], in_=recip[:, :])
                recip3 = grp.tile([P, GT], f32, tag="recip3")
                nc.vector.tensor_scalar(out=recip3[:, :], in0=recip[:, :],
                                        scalar1=0.3, scalar2=None,
                                        op0=alu.mult)
                recip2 = grp.tile([P, GT], f32, tag="recip2")
                nc.vector.tensor_scalar(out=recip2[:, :], in0=recip[:, :],
                                        scalar1=0.2, scalar2=None,
                                        op0=alu.mult)

                Pn3 = grp.tile([P, GT], f32, tag="Pn3")
                nc.vector.tensor_mul(
                    Pn3[:, :].rearrange("p (t i) -> p t i", t=GRP),
                    inflG[:, :, 0, :],
                    recip3[:, :].rearrange("p (t i) -> p t i", t=GRP))
                Gn2 = grp.tile([P, GT], f32, tag="Gn2")
                nc.vector.tensor_mul(
                    Gn2[:, :].rearrange("p (t i) -> p t i", t=GRP),
                    inflG[:, :, 1, :],
                    recip2[:, :].rearrange("p (t i) -> p t i", t=GRP))
                Mx2 = grp.tile([P, GT], f32, tag="Mx2")
                nc.vector.tensor_mul(
                    Mx2[:, :].rearrange("p (t i) -> p t i", t=GRP),
                    inflG[:, :, 2, :],
                    recip2[:, :].rearrange("p (t i) -> p t i", t=GRP))

                hE2, hP2, hG2, hL2 = (hcG[:, c, :, :].rearrange(
                    "p t j -> p (t j)") for c in range(NCH))
                st2 = stG[:, :, :].rearrange("p t i -> p (t i)")
                ho2 = [hocG[:, c, :, :].rearrange("p t i -> p (t i)")
                       for c in range(NCH)]
                tmp = grp.tile([P, GT], f32, tag="tmp")
                tmp2 = grp.tile([P, GT], f32, tag="tmp2")
                t_ = tmp[:, :]
                t2_ = tmp2[:, :]

                # E_new = clip(E + 0.3 stim - 0.4 P - 0.2 G)
                stt(nc.vector, t_, st2, 0.3, hE2)
                stt(nc.vector, t_, hP2, -0.4, t_)
                stt(nc.vector, t_, hG2, -0.2, t_)
                clip01(nc.vector, ho2[0], t_)
                # P_new = clip(0.7 P + 0.5 stim + Pn3 - 0.2 E)
                stt(nc.vector, t_, hP2, 0.7, Pn3[:, :])
                stt(nc.vector, t_, hE2, -0.2, t_)
                stt(nc.vector, t_, st2, 0.5, t_)
                clip01(nc.vector, ho2[1], t_)
                # G_new = clip(0.8 G + 0.4 E - 0.4 E*P - 0.3 P + Gn2)
                nc.vector.tensor_mul(t2_, hE2, hP2)
                stt(nc.vector, t_, t2_, -0.4, Gn2[:, :])
                stt(nc.vector, t_, hP2, -0.3, t_)
                stt(nc.vector, t_, hG2, 0.8, t_)
                stt(nc.vector, t_, hE2, 0.4, t_)
                clip01(nc.vector, ho2[2], t_)
                # L_new = clip(0.7 L - 0.3 P + Mx2)
                stt(nc.vector, t_, hL2, 0.7, Mx2[:, :])
                stt(nc.vector, t_, hP2, -0.3, t_)
                clip01(nc.vector, ho2[3], t_)

                # Lh = 0.05 * L_new (group-wide)
                nc.vector.tensor_scalar(
                    out=LhG[:, :, :].rearrange("p t i -> p (t i)"),
                    in0=ho2[3], scalar1=0.05, scalar2=None, op0=alu.mult)

                # ---- per-tile pairwise distance + W update ----
                for k in range(GRP):
                    r0 = g0 + k * TILE
                    nc.sync.dma_start(out=hoD[r0:r0 + P, :],
                                      in_=houtG[:, k, :])

                    d01 = big.tile([P, 2, NB, NB], f32, tag="d01")
                    d23 = big.tile([P, 2, NB, NB], f32, tag="d23")
                    for cset, dt_, eng in ((0, d01, nc.vector),
                                           (2, d23, nc.gpsimd)):
                        for kk in range(2):
                            c = cset + kk
                            xv = hocG[:, c, k, :]
                            xi = xv.unsqueeze(2).broadcast_to((P, NB, NB))
                            xj = xv.unsqueeze(1).broadcast_to((P, NB, NB))
                            eng.tensor_sub(dt_[:, kk, :, :], xi, xj)

                    # squares in place on ACT
                    nc.scalar.activation(out=d01[:, :, :, :],
                                         in_=d01[:, :, :, :],
                                         func=act.Square)
                    nc.scalar.activation(out=d23[:, :, :, :],
                                         in_=d23[:, :, :, :],
                                         func=act.Square)

                    # sq = (q0+q1) + (q2+q3), accumulated into d01[:,0]
                    nc.vector.tensor_add(d01[:, 0, :, :], d01[:, 0, :, :],
                                         d01[:, 1, :, :])
                    nc.gpsimd.tensor_add(d23[:, 0, :, :], d23[:, 0, :, :],
                                         d23[:, 1, :, :])
                    nc.vector.tensor_add(d01[:, 0, :, :], d01[:, 0, :, :],
                                         d23[:, 0, :, :])
                    # dist in place
                    nc.scalar.activation(out=d01[:, 0, :, :],
                                         in_=d01[:, 0, :, :], func=act.Sqrt)

                    # ml = Lh_i + Lh_j ; t2 = ml * dist (into d23[:,0])
                    Lv = LhG[:, k, :]
                    Li = Lv.unsqueeze(2).broadcast_to((P, NB, NB))
                    Lj = Lv.unsqueeze(1).broadcast_to((P, NB, NB))
                    ml = d23[:, 1, :, :]
                    nc.gpsimd.tensor_add(ml, Li, Lj)
                    nc.gpsimd.tensor_mul(d23[:, 0, :, :], ml,
                                         d01[:, 0, :, :])

                    # W_new = clip(0.95*Wz + t2), in place over Wz
                    Wt = Wts[k]
                    W3 = Wt[:, :].rearrange("p (i j) -> p i j", i=NB)
                    nc.vector.scalar_tensor_tensor(
                        out=W3, in0=W3, scalar=0.95, in1=d23[:, 0, :, :],
                        op0=alu.mult, op1=alu.add)
                    clip01(nc.vector, Wt[:, :], Wt[:, :])
                    nc.sync.dma_start(out=woD[r0:r0 + P, :], in_=Wt[:, :])

    _split_excess_waits(nc, mybir)
    return nc


def _split_excess_waits(nc, mybir, max_waits=1):
    """This walrus build encodes at most one sync-wait command per
    instruction. Hoist excess waits onto same-engine InstNoOp carriers
    inserted just before the offending instruction (engines execute their
    streams in order, so semantics are unchanged)."""
    carrier_id = [0]
    for fn in nc.m.functions:
        for blk in fn.blocks:
            insts = blk.instructions
            out = []
            changed = False
            for inst in insts:
                si = inst.sync_info
                waits = list(si.on_wait) if si and si.on_wait else []
                if len(waits) > max_waits:
                    changed = True
                    keep = waits[-max_waits:]
                    for w in waits[:-max_waits]:
                        carrier_id[0] += 1
                        nop = mybir.InstNoOp(
                            name=f"I-waitcarrier-{carrier_id[0]}", ins=[],
                            outs=[])
                        nop.engine = inst.engine
                        nop.sync_info = mybir.SyncInfo(on_wait=[w],
                                                       on_update=[])
                        out.append(nop)
                    si.on_wait = keep
                out.append(inst)
            if changed:
                blk.instructions = out


def _get_nc():
    if "nc" not in _compiled:
        _compiled["nc"] = _build()
    return _compiled["nc"]


def kernel(h, W, stim):
    from concourse.bass_utils import run_bass_kernel_spmd

    h = np.ascontiguousarray(h, dtype=np.float32)
    W = np.ascontiguousarray(W, dtype=np.float32)
    stim = np.ascontiguousarray(stim, dtype=np.float32)

    nc = _get_nc()
    in_maps = [
        {
            "h": h[c].reshape(TOK_PER_CORE, HB),
            "W": W[c].reshape(TOK_PER_CORE, NB * NB),
            "stim": stim[c].reshape(TOK_PER_CORE, NB),
        }
        for c in range(N_CORES)
    ]
    res = run_bass_kernel_spmd(nc, in_maps, list(range(N_CORES)))
    h_new = np.stack([
        res.results[c]["h_new"].reshape(S, NB, NCH) for c in range(N_CORES)
    ])
    W_new = np.stack([
        res.results[c]["W_new"].reshape(S, NB, NB) for c in range(N_CORES)
    ])
    return h_new, W_new


# revision 11
# speedup vs baseline: 1.0424x; 1.0150x over previous
"""Trainium2 Bass kernel for nn_BioTokenMucusSim (gnn_message_passing).

Layout: data-parallel over 8 cores (batch row per core). Per core, 2048
tokens are processed in 16 tiles of 128 tokens; tokens map to SBUF
partitions, each token's [32,32] W tile / [32,4] h tile lives along the
free dimension. Tiles are processed in groups of 4 so the small
per-[128,32] elementwise chains run once per group on [128,128] views.

Math (per token, nb=32):
  Wz = W with zeroed diagonal
  inflow_c = sum_j Wz[i,j] * h[j,c]   (channels P, G, mix=E+G+1.5L)
  tot = sum_j Wz[i,j]; r = 1/(tot+1e-8)
  h_new channel updates (clamped to [0,1])
  dist[i,j] = sqrt(sum_c (x_c[i]-x_c[j])^2) over the 4 new channels
  W_new = clip(0.95*Wz + (0.05*L_i + 0.05*L_j)*dist, 0, 1) (diag stays 0)

Engines: DVE = products/reduces/affine chains, ACT = squares/sqrt,
GpSimd = diffs/broadcast adds, SP = DMA.
"""
import numpy as np

B, S, NB, NCH = 8, 2048, 32, 4
TOK_PER_CORE = S
TILE = 128
NTILES = TOK_PER_CORE // TILE  # 16
GRP = 4                        # tiles per small-op group
NGRP = NTILES // GRP
N_CORES = 8
HB = NB * NCH                  # 128 floats of h per token

_compiled = {}


def _build():
    import concourse.bass as bass
    import concourse.mybir as mybir
    from concourse.tile import TileContext

    f32 = mybir.dt.float32
    alu = mybir.AluOpType
    act = mybir.ActivationFunctionType

    nc = bass.Bass("TRN2", target_bir_lowering=False, debug=False,
                   num_devices=N_CORES)
    hD = nc.declare_dram_parameter("h", [TOK_PER_CORE, HB], f32,
                                   isOutput=False)
    WD = nc.declare_dram_parameter("W", [TOK_PER_CORE, NB * NB], f32,
                                   isOutput=False)
    sD = nc.declare_dram_parameter("stim", [TOK_PER_CORE, NB], f32,
                                   isOutput=False)
    hoD = nc.declare_dram_parameter("h_new", [TOK_PER_CORE, HB], f32,
                                    isOutput=True)
    woD = nc.declare_dram_parameter("W_new", [TOK_PER_CORE, NB * NB], f32,
                                    isOutput=True)

    P = TILE

    def stt(eng, out, in0, scalar, in1):
        eng.scalar_tensor_tensor(out=out, in0=in0, scalar=float(scalar),
                                 in1=in1, op0=alu.mult, op1=alu.add)

    def clip01(eng, out, in0):
        eng.tensor_scalar(out=out, in0=in0, scalar1=0.0, scalar2=1.0,
                          op0=alu.max, op1=alu.min)

    with TileContext(nc) as tc:
        with (
            tc.tile_pool(name="wio", bufs=8) as wio,
            tc.tile_pool(name="grp", bufs=3) as grp,
            tc.tile_pool(name="big", bufs=4) as big,
        ):
            for g in range(NGRP):
                g0 = g * GRP * TILE

                # ---- group-wide tiles ----
                hG = grp.tile([P, GRP, HB], f32, tag="hG")
                stG = grp.tile([P, GRP, NB], f32, tag="stG")
                houtG = grp.tile([P, GRP, HB], f32, tag="houtG")
                inflG = grp.tile([P, GRP, 3, NB], f32, tag="inflG")
                totG = grp.tile([P, GRP, NB], f32, tag="totG")
                LhG = grp.tile([P, GRP, NB], f32, tag="LhG")
                # per-group channel views of h: [p, t, c, j] with c selected
                hcG = hG[:, :, :].rearrange("p t (j c) -> p c t j", c=NCH)
                hEg, hPg, hGg, hLg = (hcG[:, c, :, :] for c in range(NCH))
                # h_new channel views: [p, t, c, i]
                hocG = houtG[:, :, :].rearrange("p t (i c) -> p c t i", c=NCH)

                Wts = []
                for k in range(GRP):
                    r0 = g0 + k * TILE
                    Wt = wio.tile([P, NB * NB], f32, tag="Wt")
                    Wts.append(Wt)
                    nc.sync.dma_start(out=Wt[:, :], in_=WD[r0:r0 + P, :])
                    nc.sync.dma_start(out=hG[:, k, :], in_=hD[r0:r0 + P, :])
                    nc.sync.dma_start(out=stG[:, k, :], in_=sD[r0:r0 + P, :])

                # mix = E + G + 1.5 L, group-wide [p, (t j)]
                hmixG = grp.tile([P, GRP, NB], f32, tag="hmixG")
                stt(nc.vector, hmixG[:, :, :], hLg, 1.5, hGg)
                nc.vector.tensor_add(hmixG[:, :, :], hmixG[:, :, :], hEg)

                for k in range(GRP):
                    Wt = Wts[k]
                    Wflat = Wt[:, :]
                    W3 = Wflat.rearrange("p (i j) -> p i j", i=NB)
                    diag = bass.AP(tensor=Wflat.tensor, offset=Wflat.offset,
                                   ap=[list(Wflat.ap[0]), [NB + 1, NB]])
                    nc.gpsimd.memset(diag, 0.0)

                    # products for channels (P, G, mix) in one tile
                    prod3 = big.tile([P, 3, NB, NB], f32, tag="prod3")
                    hPGpack = big.tile([P, 2, NB], f32, tag="hPGpack")
                    nc.vector.tensor_copy(out=hPGpack[:, 0, :], in_=hPg[:, k, :])
                    nc.vector.tensor_copy(out=hPGpack[:, 1, :], in_=hGg[:, k, :])
                    hPGb = (hPGpack[:, :, :].unsqueeze(2)
                            .broadcast_to((P, 2, NB, NB)))
                    W4 = W3.unsqueeze(1).broadcast_to((P, 2, NB, NB))
                    nc.vector.tensor_mul(prod3[:, 0:2, :, :], W4, hPGb)
                    hMb = hmixG[:, k, :].unsqueeze(1).broadcast_to((P, NB, NB))
                    nc.gpsimd.tensor_mul(prod3[:, 2, :, :], W3, hMb)

                    nc.vector.tensor_reduce(out=inflG[:, k, :, :],
                                            in_=prod3[:, :, :, :],
                                            axis=mybir.AxisListType.X,
                                            op=alu.add)
                    nc.vector.tensor_reduce(out=totG[:, k, :], in_=W3,
                                            axis=mybir.AxisListType.X,
                                            op=alu.add)

                # ---- group-wide neighbor means + h updates ([p, GRP*NB]) ----
                GT = GRP * NB
                tot2 = totG[:, :, :].rearrange("p t i -> p (t i)")
                recip = grp.tile([P, GT], f32, tag="recip")
                nc.vector.tensor_scalar(out=recip[:, :], in0=tot2,
                                        scalar1=1e-8, scalar2=None,
                                        op0=alu.add)
                nc.vector.reciprocal(out=recip[:, :], in_=recip[:, :])
                recip3 = grp.tile([P, GT], f32, tag="recip3")
                nc.vector.tensor_scalar(out=recip3[:, :], in0=recip[:, :],
                                        scalar1=0.3, scalar2=None,
                                        op0=alu.mult)
                recip2 = grp.tile([P, GT], f32, tag="recip2")
                nc.vector.tensor_scalar(out=recip2[:, :], in0=recip[:, :],
                                        scalar1=0.2, scalar2=None,
                                        op0=alu.mult)

                Pn3 = grp.tile([P, GT], f32, tag="Pn3")
                nc.vector.tensor_mul(
                    Pn3[:, :].rearrange("p (t i) -> p t i", t=GRP),
                    inflG[:, :, 0, :],
                    recip3[:, :].rearrange("p (t i) -> p t i", t=GRP))
                Gn2 = grp.tile([P, GT], f32, tag="Gn2")
                nc.vector.tensor_mul(
                    Gn2[:, :].rearrange("p (t i) -> p t i", t=GRP),
                    inflG[:, :, 1, :],
                    recip2[:, :].rearrange("p (t i) -> p t i", t=GRP))
                Mx2 = grp.tile([P, GT], f32, tag="Mx2")
                nc.vector.tensor_mul(
                    Mx2[:, :].rearrange("p (t i) -> p t i", t=GRP),
                    inflG[:, :, 2, :],
                    recip2[:, :].rearrange("p (t i) -> p t i", t=GRP))

                hE2, hP2, hG2, hL2 = (hcG[:, c, :, :].rearrange(
                    "p t j -> p (t j)") for c in range(NCH))
                st2 = stG[:, :, :].rearrange("p t i -> p (t i)")
                ho2 = [hocG[:, c, :, :].rearrange("p t i -> p (t i)")
                       for c in range(NCH)]
                tEt = grp.tile([P, GT], f32, tag="tE")
                tPt = grp.tile([P, GT], f32, tag="tP")
                tGt = grp.tile([P, GT], f32, tag="tG")
                tLt = grp.tile([P, GT], f32, tag="tL")
                tE, tP, tG, tL = tEt[:, :], tPt[:, :], tGt[:, :], tLt[:, :]

                # E_new = clip(E + 0.3 stim - 0.4 P - 0.2 G)
                stt(nc.vector, tE, st2, 0.3, hE2)
                stt(nc.vector, tE, hP2, -0.4, tE)
                stt(nc.vector, tE, hG2, -0.2, tE)
                clip01(nc.vector, ho2[0], tE)
                # P_new = clip(0.7 P + 0.5 stim + Pn3 - 0.2 E)  [DVE]
                stt(nc.vector, tP, hP2, 0.7, Pn3[:, :])
                stt(nc.vector, tP, hE2, -0.2, tP)
                stt(nc.vector, tP, st2, 0.5, tP)
                clip01(nc.vector, ho2[1], tP)
                # G_new = clip(0.8 G + 0.4 E - 0.4 E*P - 0.3 P + Gn2)
                nc.gpsimd.tensor_mul(tG, hE2, hP2)
                stt(nc.vector, tG, tG, -0.4, Gn2[:, :])
                stt(nc.vector, tG, hP2, -0.3, tG)
                stt(nc.vector, tG, hG2, 0.8, tG)
                stt(nc.vector, tG, hE2, 0.4, tG)
                clip01(nc.vector, ho2[2], tG)
                # L_new = clip(0.7 L - 0.3 P + Mx2)  [DVE]
                stt(nc.vector, tL, hL2, 0.7, Mx2[:, :])
                stt(nc.vector, tL, hP2, -0.3, tL)
                clip01(nc.vector, ho2[3], tL)

                # Lh = 0.05 * L_new (group-wide)
                nc.vector.tensor_scalar(
                    out=LhG[:, :, :].rearrange("p t i -> p (t i)"),
                    in0=ho2[3], scalar1=0.05, scalar2=None, op0=alu.mult)

                # ---- per-tile pairwise distance + W update ----
                # All (i,j) tensors here are symmetric, so only the top
                # 16 rows (full width) and the lower-right 16x16 block are
                # computed; the lower-left block of t2 is read back through
                # a transposed view (bit-exact by symmetry).
                HF = NB // 2

                def reg_top(x3):
                    return x3[:, 0:HF, :]

                def reg_b11(x3):
                    return x3[:, HF:NB, HF:NB]

                for k in range(GRP):
                    r0 = g0 + k * TILE
                    nc.sync.dma_start(out=hoD[r0:r0 + P, :],
                                      in_=houtG[:, k, :])

                    d01 = big.tile([P, 2, NB, NB], f32, tag="d01")
                    d23 = big.tile([P, 2, NB, NB], f32, tag="d23")
                    for cset, dt_, eng in ((0, d01, nc.vector),
                                           (2, d23, nc.gpsimd)):
                        for kk in range(2):
                            c = cset + kk
                            xv = hocG[:, c, k, :]
                            xit = (xv[:, 0:HF].unsqueeze(2)
                                   .broadcast_to((P, HF, NB)))
                            xjt = xv.unsqueeze(1).broadcast_to((P, HF, NB))
                            eng.tensor_sub(reg_top(dt_[:, kk]), xit, xjt)
                            xib = (xv[:, HF:NB].unsqueeze(2)
                                   .broadcast_to((P, HF, HF)))
                            xjb = (xv[:, HF:NB].unsqueeze(1)
                                   .broadcast_to((P, HF, HF)))
                            eng.tensor_sub(reg_b11(dt_[:, kk]), xib, xjb)

                    # squares in place on ACT (both channels per instr)
                    for dt_ in (d01, d23):
                        nc.scalar.activation(out=dt_[:, :, 0:HF, :],
                                             in_=dt_[:, :, 0:HF, :],
                                             func=act.Square)
                        nc.scalar.activation(out=dt_[:, :, HF:NB, HF:NB],
                                             in_=dt_[:, :, HF:NB, HF:NB],
                                             func=act.Square)

                    # sq = (q0+q1) + (q2+q3), accumulated into d01[:,0]
                    for reg in (reg_top, reg_b11):
                        nc.vector.tensor_add(reg(d01[:, 0]), reg(d01[:, 0]),
                                             reg(d01[:, 1]))
                        nc.gpsimd.tensor_add(reg(d23[:, 0]), reg(d23[:, 0]),
                                             reg(d23[:, 1]))
                        nc.vector.tensor_add(reg(d01[:, 0]), reg(d01[:, 0]),
                                             reg(d23[:, 0]))
                    # dist in place
                    nc.scalar.activation(out=d01[:, 0, 0:HF, :],
                                         in_=d01[:, 0, 0:HF, :],
                                         func=act.Sqrt)
                    nc.scalar.activation(out=d01[:, 0, HF:NB, HF:NB],
                                         in_=d01[:, 0, HF:NB, HF:NB],
                                         func=act.Sqrt)

                    # ml = Lh_i + Lh_j (into d23[:,1]); t2 = ml*dist (d23[:,0])
                    Lv = LhG[:, k, :]
                    ml = d23[:, 1]
                    Lit = Lv[:, 0:HF].unsqueeze(2).broadcast_to((P, HF, NB))
                    Ljt = Lv.unsqueeze(1).broadcast_to((P, HF, NB))
                    nc.gpsimd.tensor_add(reg_top(ml), Lit, Ljt)
                    Lib = Lv[:, HF:NB].unsqueeze(2).broadcast_to((P, HF, HF))
                    Ljb = Lv[:, HF:NB].unsqueeze(1).broadcast_to((P, HF, HF))
                    nc.gpsimd.tensor_add(reg_b11(ml), Lib, Ljb)
                    nc.gpsimd.tensor_mul(reg_top(d23[:, 0]), reg_top(ml),
                                         reg_top(d01[:, 0]))
                    nc.gpsimd.tensor_mul(reg_b11(d23[:, 0]), reg_b11(ml),
                                         reg_b11(d01[:, 0]))

                    # W_new = clip(0.95*Wz + t2), in place over Wz;
                    # lower-left block reads t2 transposed.
                    Wt = Wts[k]
                    W3 = Wt[:, :].rearrange("p (i j) -> p i j", i=NB)
                    t2f = d23[:, 0]
                    nc.vector.scalar_tensor_tensor(
                        out=reg_top(W3), in0=reg_top(W3), scalar=0.95,
                        in1=reg_top(t2f), op0=alu.mult, op1=alu.add)
                    nc.vector.scalar_tensor_tensor(
                        out=reg_b11(W3), in0=reg_b11(W3), scalar=0.95,
                        in1=reg_b11(t2f), op0=alu.mult, op1=alu.add)
                    t2T = t2f.transpose([0, 2, 1])
                    nc.vector.scalar_tensor_tensor(
                        out=W3[:, HF:NB, 0:HF], in0=W3[:, HF:NB, 0:HF],
                        scalar=0.95, in1=t2T[:, HF:NB, 0:HF],
                        op0=alu.mult, op1=alu.add)
                    clip01(nc.vector, Wt[:, :], Wt[:, :])
                    nc.sync.dma_start(out=woD[r0:r0 + P, :], in_=Wt[:, :])

    _split_excess_waits(nc, mybir)
    return nc


def _split_excess_waits(nc, mybir, max_waits=1):
    """This walrus build encodes at most one sync-wait command per
    instruction. Hoist excess waits onto same-engine InstNoOp carriers
    inserted just before the offending instruction (engines execute their
    streams in order, so semantics are unchanged)."""
    carrier_id = [0]
    for fn in nc.m.functions:
        for blk in fn.blocks:
            insts = blk.instructions
            out = []
            changed = False
            for inst in insts:
                si = inst.sync_info
                waits = list(si.on_wait) if si and si.on_wait else []
                if len(waits) > max_waits:
                    changed = True
                    keep = waits[-max_waits:]
                    for w in waits[:-max_waits]:
                        carrier_id[0] += 1
                        nop = mybir.InstNoOp(
                            name=f"I-waitcarrier-{carrier_id[0]}", ins=[],
                            outs=[])
                        nop.engine = inst.engine
                        nop.sync_info = mybir.SyncInfo(on_wait=[w],
                                                       on_update=[])
                        out.append(nop)
                    si.on_wait = keep
                out.append(inst)
            if changed:
                blk.instructions = out


def _get_nc():
    if "nc" not in _compiled:
        _compiled["nc"] = _build()
    return _compiled["nc"]


def kernel(h, W, stim):
    from concourse.bass_utils import run_bass_kernel_spmd

    h = np.ascontiguousarray(h, dtype=np.float32)
    W = np.ascontiguousarray(W, dtype=np.float32)
    stim = np.ascontiguousarray(stim, dtype=np.float32)

    nc = _get_nc()
    in_maps = [
        {
            "h": h[c].reshape(TOK_PER_CORE, HB),
            "W": W[c].reshape(TOK_PER_CORE, NB * NB),
            "stim": stim[c].reshape(TOK_PER_CORE, NB),
        }
        for c in range(N_CORES)
    ]
    res = run_bass_kernel_spmd(nc, in_maps, list(range(N_CORES)))
    h_new = np.stack([
        res.results[c]["h_new"].reshape(S, NB, NCH) for c in range(N_CORES)
    ])
    W_new = np.stack([
        res.results[c]["W_new"].reshape(S, NB, NB) for c in range(N_CORES)
    ])
    return h_new, W_new
